# revision 14
# baseline (speedup 1.0000x reference)
"""GCN encoder (2-layer, mu/logstd heads) on 8 Trainium2 NeuronCores.

Strategy v2 (aggregate-then-project, 1D dst-partitioned graph):
  - Host: add self-loops, fold the FULL symmetric normalization into per-edge
    weights (ew = deg^-1/2[s] * w * deg^-1/2[d], f64), build a load-balancing
    node permutation (deal by in-degree into 392 blocks of 128 lanes over 8
    cores), and ONE shared edge layout used by both passes: every edge slotted
    into (core, block, lo/hi, tile, partition); wrapped-16 int16 SWDGE gather
    indices against PERMUTED node tables; per-slot dst-lane + edge-weight.
    Stage xperm = x rows permuted to table order (bf16).
  - Device (single SPMD program, TileContext):
      Pass 1: per window: dma_gather RAW x rows (bf16) -> edge-major tiles;
              omega[e,n] = (iota==dst_lane)*ew via one dual-op tensor_scalar
              per tile; PE matmuls accumulate agg[n,:] = sum_e ew*x[s] per
              128-node block in PSUM; then per block PROJECT ONCE:
              transpose(agg) -> matmul W1 -> +b1 -> ReLU -> h1 slab (SBUF,
              node-major); DMA h1 block to ag_in as soon as it's ready.
      AllGather the 8 h1 shards -> table2 (bf16, node-major, permuted order).
      Pass 2: identical windows/slots/omega against table2; per block:
              project agg2 by [Wmu||Wls] + bias, f32 out, split mu/ls DMA.
  - Host: inverse-permute rows, return (mu, logstd).

All normalization lives in ew; no per-node scaling on device. Both passes
share one index/dst/ew slab set (identical gather geometry, IN_CH==HID).
"""

import os
import sys

sys.path.insert(0, "/opt/trn_rl_repo")

import numpy as np
import ml_dtypes
from contextlib import ExitStack

import concourse.bass as bass
import concourse.bacc as bacc
import concourse.mybir as mybir
import concourse.tile as tile
from concourse.bass_utils import run_bass_kernel_spmd

P = 128
NCORES = 8
VLO = 32768          # int16 index range per gather table view
WINDOW_BLOCKS = int(os.environ.get("KERNEL_WB", "4"))

BF16 = mybir.dt.bfloat16
F32 = mybir.dt.float32
I16 = mybir.dt.int16
NPBF16 = ml_dtypes.bfloat16


def _ceil_div(a, b):
    return -(-a // b)


# ----------------------------------------------------------------------------
# Host preprocessing
# ----------------------------------------------------------------------------

def _build_pass_layout(src_rows, e_core, e_brow, e_lane, e_ew, nblk, n_table_rows,
                       vlo=VLO):
    """Slot every edge into (core, block, class, tile, partition); produce
    wrapped-16 int16 index slabs and per-slot dst-lane / edge-weight."""
    n_edges = len(src_rows)
    is_lo = src_rows < vlo
    gid = (e_core * nblk + e_brow) * 2 + (~is_lo).astype(np.int64)
    # secondary sort by src row: consecutive gather descriptors hit ascending
    # table addresses (DRAM row-buffer locality)
    order = np.argsort(gid * (1 << 17) + src_rows, kind="stable")
    gid_s = gid[order]
    counts = np.bincount(gid_s, minlength=NCORES * nblk * 2)
    starts = np.concatenate([[0], np.cumsum(counts)[:-1]])
    rank = np.arange(n_edges) - starts[gid_s]

    cnt_lo = counts[0::2].reshape(NCORES, nblk)
    cnt_hi = counts[1::2].reshape(NCORES, nblk)
    K_LO = max(1, int(_ceil_div(cnt_lo.max(), P)))
    K_HI = int(_ceil_div(cnt_hi.max(), P)) if cnt_hi.max() > 0 else 0
    K = K_LO + K_HI

    windows = []
    b = 0
    while b < nblk:
        wb = min(WINDOW_BLOCKS, nblk - b)
        windows.append((b, wb))
        b += wb

    # global tile index: window w holds [lo tiles of its wb blocks][hi tiles]
    tile_base = np.zeros(nblk, np.int64)
    win_of_brow = np.zeros(nblk, np.int64)
    j_of_brow = np.zeros(nblk, np.int64)
    wb_of_brow = np.zeros(nblk, np.int64)
    base = 0
    for w, (b0, wb) in enumerate(windows):
        for j in range(wb):
            tile_base[b0 + j] = base
            win_of_brow[b0 + j] = w
            j_of_brow[b0 + j] = j
            wb_of_brow[b0 + j] = wb
        base += wb * K
    TOT_TILES = base

    e_core_s = e_core[order]
    e_brow_s = e_brow[order]
    e_lane_s = e_lane[order]
    e_ew_s = e_ew[order]
    src_s = src_rows[order]
    is_lo_s = is_lo[order]

    k_local = rank // P
    p_slot = rank % P
    wb_s = wb_of_brow[e_brow_s]
    j_s = j_of_brow[e_brow_s]
    t_in_w = np.where(is_lo_s, j_s * K_LO + k_local,
                      wb_s * K_LO + j_s * K_HI + k_local)
    gt = tile_base[e_brow_s] + t_in_w

    dst_slab = np.full((NCORES, P, TOT_TILES), -1.0, np.float32)
    ew_slab = np.zeros((NCORES, P, TOT_TILES), np.float32)
    idx32_slab = np.zeros((NCORES, P, TOT_TILES), np.int32)
    dst_slab[e_core_s, p_slot, gt] = e_lane_s.astype(np.float32)
    ew_slab[e_core_s, p_slot, gt] = e_ew_s.astype(np.float32)
    idx32_slab[e_core_s, p_slot, gt] = src_s.astype(np.int32)

    lo_cols_per_win = [wb * K_LO * P // 16 for (_, wb) in windows]
    hi_cols_per_win = [wb * K_HI * P // 16 for (_, wb) in windows]
    lo_col_base = np.concatenate([[0], np.cumsum(lo_cols_per_win)[:-1]]).astype(np.int64)
    hi_col_base = np.concatenate([[0], np.cumsum(hi_cols_per_win)[:-1]]).astype(np.int64)
    lo_idx = np.zeros((NCORES, 16, int(sum(lo_cols_per_win))), np.int16)
    hi_idx = np.zeros((NCORES, 16, max(1, int(sum(hi_cols_per_win)))), np.int16)

    flat_in_region = np.where(
        is_lo_s,
        (j_s * K_LO + k_local) * P + p_slot,
        (j_s * K_HI + k_local) * P + p_slot,
    )
    w_s = win_of_brow[e_brow_s]
    col = np.where(is_lo_s, lo_col_base[w_s], hi_col_base[w_s]) + flat_in_region // 16
    row = flat_in_region % 16
    lo_mask = is_lo_s
    lo_idx[e_core_s[lo_mask], row[lo_mask], col[lo_mask]] = src_s[lo_mask].astype(np.int16)
    if K_HI > 0:
        hi_mask = ~is_lo_s
        hi_idx[e_core_s[hi_mask], row[hi_mask], col[hi_mask]] = (
            (src_s[hi_mask] - vlo).astype(np.int16))

    return dict(
        VLO=vlo,
        K_LO=K_LO, K_HI=K_HI, K=K, TOT_TILES=TOT_TILES, windows=windows,
        dst_slab=dst_slab, ew_slab=ew_slab, idx32_slab=idx32_slab,
        lo_idx=np.tile(lo_idx, (1, 8, 1)), hi_idx=np.tile(hi_idx, (1, 8, 1)),
        lo_col_base=lo_col_base, hi_col_base=hi_col_base,
        n_table_rows=n_table_rows,
    )


def _preprocess(x, edge_index, weight):
    N = x.shape[0]
    s = edge_index[0].astype(np.int64)
    d = edge_index[1].astype(np.int64)
    w = weight.astype(np.float64)
    s = np.concatenate([s, np.arange(N)])
    d = np.concatenate([d, np.arange(N)])
    w = np.concatenate([w, np.ones(N)])

    deg = np.bincount(d, weights=w, minlength=N)
    dis = np.where(deg > 0, deg ** -0.5, 0.0)
    ew = dis[s] * w * dis[d]          # full symmetric norm folded into ew

    NB = NCORES * _ceil_div(_ceil_div(N, NCORES), P)
    nblk = NB // NCORES
    PAD_CORE = nblk * P
    PAD_N = NB * P

    # balance: round-robin deal nodes (sorted by in-degree desc) into NB blocks
    tot = np.bincount(d, minlength=N)
    order = np.argsort(-tot, kind="stable")
    blk = np.empty(N, np.int64)
    lane = np.empty(N, np.int64)
    blk[order] = np.arange(N) % NB
    lane[order] = np.arange(N) // NB
    assert lane.max() < P
    core_of = blk // nblk
    brow_of = blk % nblk
    permpos = core_of * PAD_CORE + brow_of * P + lane

    # chunk-major table layout so each AllGather chunk lands contiguously:
    # row(core, brow, lane) = chunk*8*CB*P + core*CB*P + (brow%CB)*P + lane
    CB = int(os.environ.get("KERNEL_CB", "7"))
    nchunk = _ceil_div(nblk, CB)
    RPC = NCORES * CB * P           # table rows per chunk
    chunk_of = brow_of // CB
    permtab = (chunk_of * RPC + core_of * CB * P
               + (brow_of % CB) * P + lane)
    PAD_T = nchunk * RPC            # padded table rows (>= PAD_N)

    e_core = core_of[d]
    e_brow = brow_of[d]
    e_lane = lane[d]

    split = os.environ.get("KERNEL_P2SPLIT", "0") == "1"
    if split:
        c_min = _ceil_div(PAD_T - VLO, RPC)
        c_max = VLO // RPC
        loch = max(c_min, min(c_max, (nchunk + 1) // 2))
        vlo = loch * RPC
    else:
        loch, vlo = 0, VLO

    pl = _build_pass_layout(permtab[s], e_core, e_brow, e_lane, ew, nblk, PAD_T,
                            vlo=vlo)

    return dict(
        N=N, NB=NB, nblk=nblk, PAD_CORE=PAD_CORE, PAD_N=PAD_N,
        permpos=permpos, permtab=permtab, CB=CB, nchunk=nchunk, RPC=RPC,
        PAD_T=PAD_T, pl=pl, split=split, LOCH=loch,
    )


# ----------------------------------------------------------------------------
# Device program
# ----------------------------------------------------------------------------

def _emit_pass(nc, pools, pl, tables, lo_s, hi_s,
               dst_s, ew_s, iota_s, flush_fn, ix32_s=None,
               phase=None, accs=None, identity_s=None):
    """phase=None: both classes, PSUM acc per block, flush_fn(brow, acc).
    phase=0: lo class only; per block copy acc into accs slab (bf16).
    phase=1: hi class only; per block seed acc from accs via identity matmul,
             then accumulate hi tiles and flush_fn(brow, acc)."""
    abl = os.environ.get("KERNEL_ABL", "")
    gmode = os.environ.get("KERNEL_GMODE", "swdge")
    TW = int(os.environ.get("KERNEL_ESZ", "128"))  # table row width (elems)
    K_LO, K_HI, K = pl["K_LO"], pl["K_HI"], pl["K"]
    vlo = pl["VLO"]
    windows = pl["windows"]
    lo_col_base, hi_col_base = pl["lo_col_base"], pl["hi_col_base"]
    rows = pl["n_table_rows"]
    msg_pool, omega_pool, psum_pool = pools["msg"], pools["omega"], pools["psum"]
    nq = int(os.environ.get("KERNEL_NSWQ", "2"))

    max_wb = max(wb for _, wb in windows)
    tbl_lo, tbl_hi = tables
    do_lo = phase in (None, 0)
    do_hi = phase in (None, 1)
    mtag, otag, mtiles = "msg", "omega", K

    for w, (b0, wb) in enumerate(windows):
        nlo_tiles = wb * K_LO if do_lo else 0
        nhi_tiles = wb * K_HI if do_hi else 0
        wtiles = nlo_tiles + nhi_tiles
        msg = msg_pool.tile([P, max_wb * mtiles, TW], BF16, tag=mtag)
        omega = omega_pool.tile([P, max_wb * mtiles * P], BF16, tag=otag)
        n_lo = wb * K_LO * P
        if "nogather" in abl:
            pass
        elif gmode == "dumb":
            # diagnostic: same bytes, contiguous stream instead of gather
            nc.sync.dma_start(
                out=msg[:, 0:wtiles, :],
                in_=tbl_lo[0:wtiles * P, :].rearrange(
                    "(a b) c -> a (b c)", a=P))
        else:
            if do_lo:
                nc.gpsimd.dma_gather(
                    out_ap=msg[:, 0:nlo_tiles, :],
                    in_ap=tbl_lo,
                    idxs_ap=lo_s[:, int(lo_col_base[w]):int(lo_col_base[w]) + n_lo // 16],
                    num_idxs=n_lo,
                    num_idxs_reg=n_lo,
                    elem_size=TW,
                    queue_num=(2 * w) % nq,
                    single_packet=(n_lo <= 1024),
                )
            if do_hi and K_HI > 0:
                n_hi = wb * K_HI * P
                nc.gpsimd.dma_gather(
                    out_ap=msg[:, nlo_tiles:nlo_tiles + wb * K_HI, :],
                    in_ap=tbl_hi,
                    idxs_ap=hi_s[:, int(hi_col_base[w]):int(hi_col_base[w]) + n_hi // 16],
                    num_idxs=n_hi,
                    num_idxs_reg=n_hi,
                    elem_size=TW,
                    queue_num=(2 * w + 1) % nq,
                    single_packet=(n_hi <= 1024),
                )
        gt0 = b0 * K
        ghi0 = gt0 + (wb * K_LO if do_hi else 0)  # global idx of first hi tile
        if "noomega" not in abl:
            for t in range(wtiles):
                gt = (gt0 + t) if t < nlo_tiles or phase is None else (
                    ghi0 + (t - nlo_tiles))
                nc.vector.tensor_scalar(
                    out=omega[:, t * P:(t + 1) * P],
                    in0=iota_s,
                    scalar1=dst_s[:, gt:gt + 1],
                    scalar2=ew_s[:, gt:gt + 1],
                    op0=mybir.AluOpType.is_equal,
                    op1=mybir.AluOpType.mult,
                )
        if "noflush" in abl and "nomm" in abl:
            continue
        for j in range(wb):
            brow = b0 + j
            acc = psum_pool.tile([P, P], F32, tag="acc", space="PSUM")
            if "nomm" not in abl:
                started = False
                if phase == 1:
                    nc.tensor.matmul(
                        out=acc[:], lhsT=identity_s,
                        rhs=accs[:, brow * P:(brow + 1) * P],
                        start=True, stop=(K_HI == 0))
                    started = True
                if do_lo:
                    for k in range(K_LO):
                        t = j * K_LO + k
                        nc.tensor.matmul(
                            out=acc[:], lhsT=omega[:, t * P:(t + 1) * P],
                            rhs=msg[:, t, 0:P], start=not started and k == 0,
                            stop=(phase is None and k == K - 1 and K_HI == 0)
                                 or (phase == 0 and k == K_LO - 1))
                    started = True
                if do_hi:
                    for k in range(K_HI):
                        t = nlo_tiles + j * K_HI + k
                        nc.tensor.matmul(
                            out=acc[:], lhsT=omega[:, t * P:(t + 1) * P],
                            rhs=msg[:, t, 0:P],
                            start=not started and k == 0,
                            stop=(k == K_HI - 1))
            else:
                nc.tensor.matmul(out=acc[:], lhsT=iota_s, rhs=iota_s,
                                 start=True, stop=True)
            if phase == 0:
                nc.scalar.copy(out=accs[:, brow * P:(brow + 1) * P], in_=acc[:])
            elif "noflush" not in abl:
                flush_fn(brow, acc)


def _build_program(meta, IN_CH, HID, OUT):
    pl = meta["pl"]
    nblk = meta["nblk"]
    PAD_CORE, PAD_N = meta["PAD_CORE"], meta["PAD_N"]
    HOUT = 2 * OUT
    abl = os.environ.get("KERNEL_ABL", "")

    nq = int(os.environ.get("KERNEL_NSWQ", "2"))
    scratch = int(os.environ.get("KERNEL_SCRATCH", "16384"))
    nc = bacc.Bacc(num_swdge_queues=nq, dynamic_dma_scratch_size=scratch)
    TW = int(os.environ.get("KERNEL_ESZ", "128"))
    xp_t = nc.declare_dram_parameter("xperm", [meta["PAD_T"], TW], BF16, isOutput=False)
    W1_t = nc.declare_dram_parameter("W1", [P, HID], BF16, isOutput=False)
    Wcat_t = nc.declare_dram_parameter("Wcat", [HID, HOUT], BF16, isOutput=False)
    b1_t = nc.declare_dram_parameter("b1", [1, HID], BF16, isOutput=False)
    bcat_t = nc.declare_dram_parameter("bcat", [1, HOUT], BF16, isOutput=False)
    iota_t = nc.declare_dram_parameter("iota", [P, P], BF16, isOutput=False)

    lo_t = nc.declare_dram_parameter("lo", [P, pl["lo_idx"].shape[2]], I16, isOutput=False)
    hi_t = nc.declare_dram_parameter("hi", [P, pl["hi_idx"].shape[2]], I16, isOutput=False)
    dst_t = nc.declare_dram_parameter("dst", [P, pl["TOT_TILES"]], F32, isOutput=False)
    ew_t = nc.declare_dram_parameter("ew", [P, pl["TOT_TILES"]], F32, isOutput=False)
    gmode = os.environ.get("KERNEL_GMODE", "swdge")
    ix32_t = (nc.declare_dram_parameter("ix32", [P, pl["TOT_TILES"]],
                                        mybir.dt.int32, isOutput=False)
              if gmode == "ind" else None)

    mu_t = nc.declare_dram_parameter("mu", [PAD_CORE, OUT], F32, isOutput=True)
    ls_t = nc.declare_dram_parameter("ls", [PAD_CORE, OUT], F32, isOutput=True)

    CB, nchunk, RPC = meta["CB"], meta["nchunk"], meta["RPC"]
    PAD_T = meta["PAD_T"]
    split, LOCH = meta["split"], meta["LOCH"]
    VLO_T = meta["pl"]["VLO"]
    t2mode = os.environ.get("KERNEL_T2", "shared")
    ag_ins = [
        nc.dram_tensor(f"ag_in{k}", [min(CB, nblk - k * CB) * P, HID], BF16)
        for k in range(nchunk)
    ]
    if split:
        t2sh_lo = nc.dram_tensor("t2shlo", [VLO_T, HID], BF16,
                                 addr_space="Shared")
        t2sh_hi = nc.dram_tensor("t2shhi", [PAD_T - VLO_T, HID], BF16,
                                 addr_space="Shared")
        t2lo = nc.dram_tensor("t2lo", [VLO_T, HID], BF16)
        t2hi = nc.dram_tensor("t2hi", [PAD_T - VLO_T, HID], BF16)
        table2 = table2_loc = None
    elif t2mode == "localout":
        table2 = nc.dram_tensor("table2", [PAD_T, HID], BF16)
        table2_loc = table2
    else:
        table2 = nc.dram_tensor("table2", [PAD_T, HID], BF16, addr_space="Shared")
        table2_loc = (nc.dram_tensor("table2loc", [PAD_T, HID], BF16)
                      if t2mode == "copy" else table2)

    with tile.TileContext(nc) as tc, ExitStack() as ctx:
        const = ctx.enter_context(tc.tile_pool(name="const", bufs=1))
        stage_pool = ctx.enter_context(tc.tile_pool(name="stage", bufs=3))
        msg_pool = ctx.enter_context(tc.tile_pool(name="msg", bufs=2))
        omega_pool = ctx.enter_context(tc.tile_pool(name="omega", bufs=2))
        psum_pool = ctx.enter_context(tc.tile_pool(name="psum", bufs=3, space="PSUM"))
        pr_pool = ctx.enter_context(tc.tile_pool(name="prpsum", bufs=2, space="PSUM"))
        tp_pool = ctx.enter_context(tc.tile_pool(name="tpsum", bufs=2, space="PSUM"))

        def load_const(param, shape, dtype):
            s = const.tile(shape, dtype, tag=param.name)
            nc.sync.dma_start(out=s[:], in_=param[:])
            return s[:]

        W1_s = load_const(W1_t, [P, HID], BF16)
        Wcat_s = load_const(Wcat_t, [HID, HOUT], BF16)
        b1_s = load_const(b1_t, [1, HID], BF16)
        bcat_s = load_const(bcat_t, [1, HOUT], BF16)
        iota_s = load_const(iota_t, [P, P], BF16)
        lo_s = load_const(lo_t, [P, pl["lo_idx"].shape[2]], I16)
        hi_s = load_const(hi_t, [P, pl["hi_idx"].shape[2]], I16)
        dst_s = load_const(dst_t, [P, pl["TOT_TILES"]], F32)
        ew_s = load_const(ew_t, [P, pl["TOT_TILES"]], F32)
        ix32_s = (load_const(ix32_t, [P, pl["TOT_TILES"]], mybir.dt.int32)
                  if ix32_t is not None else None)

        ones_s = const.tile([1, P], BF16, tag="ones")
        nc.vector.memset(ones_s[:], 1.0)
        identity_s = const.tile([P, P], BF16, tag="identity")
        nc.vector.memset(identity_s[:], 0.0)
        nc.gpsimd.affine_select(
            out=identity_s[:], in_=identity_s[:],
            compare_op=mybir.AluOpType.not_equal, fill=1.0,
            base=0, pattern=[[-1, P]], channel_multiplier=1)

        h1 = const.tile([P, nblk * HID], BF16, tag="h1")

        pools = dict(msg=msg_pool, omega=omega_pool, psum=psum_pool)

        def project_block(acc, Ws, bias_s, width):
            """PSUM agg [P,P] -> transpose -> @Ws + bias -> PSUM [P,width]."""
            c = stage_pool.tile([P, P], BF16, tag="pb_c")
            nc.scalar.copy(out=c[:], in_=acc[:])
            tp = tp_pool.tile([P, P], BF16, tag="pb_tp", space="PSUM")
            nc.tensor.transpose(out=tp[:], in_=c[:], identity=identity_s)
            cT = stage_pool.tile([P, P], BF16, tag="pb_cT")
            nc.scalar.copy(out=cT[:], in_=tp[:])
            pr = pr_pool.tile([P, width], F32, tag="pb_pr", space="PSUM")
            nc.tensor.matmul(out=pr[:], lhsT=cT[:], rhs=Ws, start=True, stop=False)
            nc.tensor.matmul(out=pr[:], lhsT=ones_s[:], rhs=bias_s,
                             start=False, stop=True)
            return pr

        def emit_chunk_ag(k):
            if "noAG" in abl:
                return
            rows_k = min(CB, nblk - k * CB) * P
            if split:
                if k < LOCH:
                    sh, loc, base = t2sh_lo, t2lo, k * RPC
                else:
                    sh, loc, base = t2sh_hi, t2hi, k * RPC - VLO_T
                nc.gpsimd.collective_compute(
                    "AllGather", mybir.AluOpType.bypass,
                    replica_groups=[list(range(NCORES))],
                    ins=[ag_ins[k][:]],
                    outs=[sh[base:base + rows_k * NCORES, :]])
                nc.sync.dma_start(
                    out=loc[base:base + rows_k * NCORES, :].rearrange(
                        "(a b) c -> a (b c)", a=P),
                    in_=sh[base:base + rows_k * NCORES, :].rearrange(
                        "(a b) c -> a (b c)", a=P))
                return
            nc.gpsimd.collective_compute(
                "AllGather", mybir.AluOpType.bypass,
                replica_groups=[list(range(NCORES))],
                ins=[ag_ins[k][:]],
                outs=[table2[k * RPC:k * RPC + rows_k * NCORES, :]])
            if t2mode == "copy":
                nc.sync.dma_start(
                    out=table2_loc[k * RPC:k * RPC + rows_k * NCORES, :].rearrange(
                        "(a b) c -> a (b c)", a=P),
                    in_=table2[k * RPC:k * RPC + rows_k * NCORES, :].rearrange(
                        "(a b) c -> a (b c)", a=P))

        def flush1(brow, acc):
            pr = project_block(acc, W1_s, b1_s, HID)
            nc.scalar.activation(out=h1[:, brow * HID:(brow + 1) * HID], in_=pr[:],
                                 func=mybir.ActivationFunctionType.Relu)
            k, r = brow // CB, brow % CB
            nc.sync.dma_start(out=ag_ins[k][r * P:(r + 1) * P, :],
                              in_=h1[:, brow * HID:(brow + 1) * HID])
            if brow == nblk - 1 or r == CB - 1:
                emit_chunk_ag(k)

        xp_tables = (xp_t[0:meta["pl"]["VLO"], :],
                     xp_t[meta["pl"]["VLO"]:PAD_T, :])
        if "noB" not in abl:
            _emit_pass(nc, pools, pl, xp_tables, lo_s, hi_s, dst_s, ew_s, iota_s, flush1, ix32_s=ix32_s)
        else:
            nc.vector.memset(h1[:], 0.1)
            for brow in range(nblk):
                k, r = brow // CB, brow % CB
                nc.sync.dma_start(out=ag_ins[k][r * P:(r + 1) * P, :],
                                  in_=h1[:, brow * HID:(brow + 1) * HID])
                if brow == nblk - 1 or r == CB - 1:
                    emit_chunk_ag(k)

        if not split:
            tc.strict_bb_all_engine_barrier()

        def flush2(brow, acc):
            pr = project_block(acc, Wcat_s, bcat_s, HOUT)
            o = stage_pool.tile([P, HOUT], F32, tag="otile")
            nc.scalar.copy(out=o[:], in_=pr[:])
            nc.sync.dma_start(out=mu_t[brow * P:(brow + 1) * P, :], in_=o[:, 0:OUT])
            nc.sync.dma_start(out=ls_t[brow * P:(brow + 1) * P, :], in_=o[:, OUT:HOUT])

        if "noD" not in abl:
            if split:
                accs = const.tile([P, nblk * P], BF16, tag="accs")
                _emit_pass(nc, pools, pl, (t2lo[:], None), lo_s, hi_s,
                           dst_s, ew_s, iota_s, None, phase=0, accs=accs)
                _emit_pass(nc, pools, pl, (None, t2hi[:]), lo_s, hi_s,
                           dst_s, ew_s, iota_s, flush2, phase=1, accs=accs,
                           identity_s=identity_s)
            else:
                # p2local: diagnostic — pass 2 from local xperm instead of
                # Shared table2 (wrong numerics, isolates Shared-gather cost)
                t2 = xp_t if "p2local" in abl else table2_loc
                t2_tables = (t2[0:meta["pl"]["VLO"], :],
                             t2[meta["pl"]["VLO"]:PAD_T, :])
                _emit_pass(nc, pools, pl, t2_tables, lo_s, hi_s, dst_s, ew_s,
                           iota_s, flush2, ix32_s=ix32_s)

    nc.finalize()
    return nc


# ----------------------------------------------------------------------------
# Public entry
# ----------------------------------------------------------------------------

def _prepare(x, edge_index, weight, W1, b1, Wmu, bmu, Wls, bls):
    x = np.asarray(x)
    N, IN_CH = x.shape
    HID = np.asarray(W1).shape[1]
    OUT = np.asarray(Wmu).shape[1]
    meta = _preprocess(x, np.asarray(edge_index), np.asarray(weight))
    pl = meta["pl"]

    nc = _build_program(meta, IN_CH, HID, OUT)

    TW = int(os.environ.get("KERNEL_ESZ", "128"))
    xperm = np.zeros((meta["PAD_T"], TW), np.float32)
    xperm[meta["permtab"], 0:IN_CH] = np.asarray(x, np.float32)
    Wcat = np.concatenate([np.asarray(Wmu), np.asarray(Wls)], axis=1)
    bcat = np.concatenate([np.asarray(bmu), np.asarray(bls)])
    iota = np.tile(np.arange(P, dtype=np.float32)[None, :], (P, 1))

    common = {
        "xperm": xperm.astype(NPBF16),
        "W1": np.asarray(W1, np.float32).astype(NPBF16),
        "Wcat": Wcat.astype(np.float32).astype(NPBF16),
        "b1": np.asarray(b1, np.float32).astype(NPBF16)[None, :],
        "bcat": bcat.astype(np.float32).astype(NPBF16)[None, :],
        "iota": iota.astype(NPBF16),
    }
    in_maps = []
    for c in range(NCORES):
        m = dict(common)
        m["lo"] = pl["lo_idx"][c]
        m["hi"] = pl["hi_idx"][c]
        if os.environ.get("KERNEL_GMODE", "swdge") == "ind":
            m["ix32"] = pl["idx32_slab"][c]
        m["dst"] = pl["dst_slab"][c]
        m["ew"] = pl["ew_slab"][c]
        in_maps.append(m)
    return nc, in_maps, meta


def _postprocess(results, meta):
    mu_cat = np.concatenate([results[c]["mu"] for c in range(NCORES)])
    ls_cat = np.concatenate([results[c]["ls"] for c in range(NCORES)])
    mu = mu_cat[meta["permpos"]].astype(np.float32)
    ls = ls_cat[meta["permpos"]].astype(np.float32)
    return mu, ls


def _run(x, edge_index, weight, W1, b1, Wmu, bmu, Wls, bls, trace=False):
    nc, in_maps, meta = _prepare(x, edge_index, weight, W1, b1, Wmu, bmu, Wls, bls)
    res = run_bass_kernel_spmd(nc, in_maps, list(range(NCORES)), trace=trace)
    return _postprocess(res.results, meta), res


def kernel(x, edge_index, weight, W1, b1, Wmu, bmu, Wls, bls):
    (mu, ls), _ = _run(x, edge_index, weight, W1, b1, Wmu, bmu, Wls, bls)
    return mu, ls



# revision 15
# speedup vs baseline: 2.9201x; 2.9201x over previous
"""GCN encoder (2-layer, mu/logstd heads) on 8 Trainium2 NeuronCores.

Strategy v2 (aggregate-then-project, 1D dst-partitioned graph):
  - Host: add self-loops, fold the FULL symmetric normalization into per-edge
    weights (ew = deg^-1/2[s] * w * deg^-1/2[d], f64), build a load-balancing
    node permutation (deal by in-degree into 392 blocks of 128 lanes over 8
    cores), and ONE shared edge layout used by both passes: every edge slotted
    into (core, block, lo/hi, tile, partition); wrapped-16 int16 SWDGE gather
    indices against PERMUTED node tables; per-slot dst-lane + edge-weight.
    Stage xperm = x rows permuted to table order (bf16).
  - Device (single SPMD program, TileContext):
      Pass 1: per window: dma_gather RAW x rows (bf16) -> edge-major tiles;
              omega[e,n] = (iota==dst_lane)*ew via one dual-op tensor_scalar
              per tile; PE matmuls accumulate agg[n,:] = sum_e ew*x[s] per
              128-node block in PSUM; then per block PROJECT ONCE:
              transpose(agg) -> matmul W1 -> +b1 -> ReLU -> h1 slab (SBUF,
              node-major); DMA h1 block to ag_in as soon as it's ready.
      AllGather the 8 h1 shards -> table2 (bf16, node-major, permuted order).
      Pass 2: identical windows/slots/omega against table2; per block:
              project agg2 by [Wmu||Wls] + bias, f32 out, split mu/ls DMA.
  - Host: inverse-permute rows, return (mu, logstd).

All normalization lives in ew; no per-node scaling on device. Both passes
share one index/dst/ew slab set (identical gather geometry, IN_CH==HID).
"""

import os
import sys

sys.path.insert(0, "/opt/trn_rl_repo")

import numpy as np
import ml_dtypes
from contextlib import ExitStack

import concourse.bass as bass
import concourse.bacc as bacc
import concourse.mybir as mybir
import concourse.tile as tile
from concourse.bass_utils import run_bass_kernel_spmd

P = 128
NCORES = 8
VLO = 32768          # int16 index range per gather table view
WINDOW_BLOCKS = int(os.environ.get("KERNEL_WB", "4"))

BF16 = mybir.dt.bfloat16
F32 = mybir.dt.float32
I16 = mybir.dt.int16
NPBF16 = ml_dtypes.bfloat16


def _ceil_div(a, b):
    return -(-a // b)


# ----------------------------------------------------------------------------
# Host preprocessing
# ----------------------------------------------------------------------------

def _build_pass_layout(src_rows, e_core, e_brow, e_lane, e_ew, nblk, n_table_rows,
                       vlo=VLO):
    """Slot every edge into (core, block, class, tile, partition); produce
    wrapped-16 int16 index slabs and per-slot dst-lane / edge-weight."""
    n_edges = len(src_rows)
    is_lo = src_rows < vlo
    gid = (e_core * nblk + e_brow) * 2 + (~is_lo).astype(np.int64)
    # secondary sort by src row: consecutive gather descriptors hit ascending
    # table addresses (DRAM row-buffer locality)
    order = np.argsort(gid * (1 << 17) + src_rows, kind="stable")
    gid_s = gid[order]
    counts = np.bincount(gid_s, minlength=NCORES * nblk * 2)
    starts = np.concatenate([[0], np.cumsum(counts)[:-1]])
    rank = np.arange(n_edges) - starts[gid_s]

    cnt_lo = counts[0::2].reshape(NCORES, nblk)
    cnt_hi = counts[1::2].reshape(NCORES, nblk)
    K_LO = max(1, int(_ceil_div(cnt_lo.max(), P)))
    K_HI = int(_ceil_div(cnt_hi.max(), P)) if cnt_hi.max() > 0 else 0
    K = K_LO + K_HI

    windows = []
    b = 0
    while b < nblk:
        wb = min(WINDOW_BLOCKS, nblk - b)
        windows.append((b, wb))
        b += wb

    # global tile index: window w holds [lo tiles of its wb blocks][hi tiles]
    tile_base = np.zeros(nblk, np.int64)
    win_of_brow = np.zeros(nblk, np.int64)
    j_of_brow = np.zeros(nblk, np.int64)
    wb_of_brow = np.zeros(nblk, np.int64)
    base = 0
    for w, (b0, wb) in enumerate(windows):
        for j in range(wb):
            tile_base[b0 + j] = base
            win_of_brow[b0 + j] = w
            j_of_brow[b0 + j] = j
            wb_of_brow[b0 + j] = wb
        base += wb * K
    TOT_TILES = base

    e_core_s = e_core[order]
    e_brow_s = e_brow[order]
    e_lane_s = e_lane[order]
    e_ew_s = e_ew[order]
    src_s = src_rows[order]
    is_lo_s = is_lo[order]

    k_local = rank // P
    p_slot = rank % P
    wb_s = wb_of_brow[e_brow_s]
    j_s = j_of_brow[e_brow_s]
    t_in_w = np.where(is_lo_s, j_s * K_LO + k_local,
                      wb_s * K_LO + j_s * K_HI + k_local)
    gt = tile_base[e_brow_s] + t_in_w

    dst_slab = np.full((NCORES, P, TOT_TILES), -1.0, np.float32)
    ew_slab = np.zeros((NCORES, P, TOT_TILES), np.float32)
    idx32_slab = np.zeros((NCORES, P, TOT_TILES), np.int32)
    dst_slab[e_core_s, p_slot, gt] = e_lane_s.astype(np.float32)
    ew_slab[e_core_s, p_slot, gt] = e_ew_s.astype(np.float32)
    idx32_slab[e_core_s, p_slot, gt] = src_s.astype(np.int32)

    lo_cols_per_win = [wb * K_LO * P // 16 for (_, wb) in windows]
    hi_cols_per_win = [wb * K_HI * P // 16 for (_, wb) in windows]
    lo_col_base = np.concatenate([[0], np.cumsum(lo_cols_per_win)[:-1]]).astype(np.int64)
    hi_col_base = np.concatenate([[0], np.cumsum(hi_cols_per_win)[:-1]]).astype(np.int64)
    lo_idx = np.zeros((NCORES, 16, int(sum(lo_cols_per_win))), np.int16)
    hi_idx = np.zeros((NCORES, 16, max(1, int(sum(hi_cols_per_win)))), np.int16)

    flat_in_region = np.where(
        is_lo_s,
        (j_s * K_LO + k_local) * P + p_slot,
        (j_s * K_HI + k_local) * P + p_slot,
    )
    w_s = win_of_brow[e_brow_s]
    col = np.where(is_lo_s, lo_col_base[w_s], hi_col_base[w_s]) + flat_in_region // 16
    row = flat_in_region % 16
    lo_mask = is_lo_s
    lo_idx[e_core_s[lo_mask], row[lo_mask], col[lo_mask]] = src_s[lo_mask].astype(np.int16)
    if K_HI > 0:
        hi_mask = ~is_lo_s
        hi_idx[e_core_s[hi_mask], row[hi_mask], col[hi_mask]] = (
            (src_s[hi_mask] - vlo).astype(np.int16))

    return dict(
        VLO=vlo,
        K_LO=K_LO, K_HI=K_HI, K=K, TOT_TILES=TOT_TILES, windows=windows,
        dst_slab=dst_slab, ew_slab=ew_slab, idx32_slab=idx32_slab,
        lo_idx=np.tile(lo_idx, (1, 8, 1)), hi_idx=np.tile(hi_idx, (1, 8, 1)),
        lo_col_base=lo_col_base, hi_col_base=hi_col_base,
        n_table_rows=n_table_rows,
    )


def _preprocess(x, edge_index, weight):
    N = x.shape[0]
    s = edge_index[0].astype(np.int64)
    d = edge_index[1].astype(np.int64)
    w = weight.astype(np.float64)
    s = np.concatenate([s, np.arange(N)])
    d = np.concatenate([d, np.arange(N)])
    w = np.concatenate([w, np.ones(N)])

    deg = np.bincount(d, weights=w, minlength=N)
    dis = np.where(deg > 0, deg ** -0.5, 0.0)
    ew = dis[s] * w * dis[d]          # full symmetric norm folded into ew

    NB = NCORES * _ceil_div(_ceil_div(N, NCORES), P)
    nblk = NB // NCORES
    PAD_CORE = nblk * P
    PAD_N = NB * P

    # balance: round-robin deal nodes (sorted by in-degree desc) into NB blocks
    tot = np.bincount(d, minlength=N)
    order = np.argsort(-tot, kind="stable")
    blk = np.empty(N, np.int64)
    lane = np.empty(N, np.int64)
    blk[order] = np.arange(N) % NB
    lane[order] = np.arange(N) // NB
    assert lane.max() < P
    core_of = blk // nblk
    brow_of = blk % nblk
    permpos = core_of * PAD_CORE + brow_of * P + lane

    # chunk-major table layout so each AllGather chunk lands contiguously:
    # row(core, brow, lane) = chunk*8*CB*P + core*CB*P + (brow%CB)*P + lane
    CB = int(os.environ.get("KERNEL_CB", "7"))
    nchunk = _ceil_div(nblk, CB)
    RPC = NCORES * CB * P           # table rows per chunk
    chunk_of = brow_of // CB
    permtab = (chunk_of * RPC + core_of * CB * P
               + (brow_of % CB) * P + lane)
    PAD_T = nchunk * RPC            # padded table rows (>= PAD_N)

    e_core = core_of[d]
    e_brow = brow_of[d]
    e_lane = lane[d]

    split = os.environ.get("KERNEL_P2SPLIT", "0") == "1"
    if split:
        c_min = _ceil_div(PAD_T - VLO, RPC)
        c_max = VLO // RPC
        loch = max(c_min, min(c_max, (nchunk + 1) // 2))
        vlo = loch * RPC
    else:
        loch, vlo = 0, VLO

    pl = _build_pass_layout(permtab[s], e_core, e_brow, e_lane, ew, nblk, PAD_T,
                            vlo=vlo)

    return dict(
        N=N, NB=NB, nblk=nblk, PAD_CORE=PAD_CORE, PAD_N=PAD_N,
        permpos=permpos, permtab=permtab, CB=CB, nchunk=nchunk, RPC=RPC,
        PAD_T=PAD_T, pl=pl, split=split, LOCH=loch,
    )


# ----------------------------------------------------------------------------
# Device program
# ----------------------------------------------------------------------------

def _emit_pass(nc, pools, pl, tables, lo_s, hi_s,
               dst_s, ew_s, iota_s, flush_fn, ix32_s=None,
               phase=None, accs=None, identity_s=None):
    """phase=None: both classes, PSUM acc per block, flush_fn(brow, acc).
    phase=0: lo class only; per block copy acc into accs slab (bf16).
    phase=1: hi class only; per block seed acc from accs via identity matmul,
             then accumulate hi tiles and flush_fn(brow, acc)."""
    abl = os.environ.get("KERNEL_ABL", "")
    gmode = os.environ.get("KERNEL_GMODE", "swdge")
    TW = int(os.environ.get("KERNEL_ESZ", "128"))  # table row width (elems)
    K_LO, K_HI, K = pl["K_LO"], pl["K_HI"], pl["K"]
    vlo = pl["VLO"]
    windows = pl["windows"]
    lo_col_base, hi_col_base = pl["lo_col_base"], pl["hi_col_base"]
    rows = pl["n_table_rows"]
    msg_pool, omega_pool, psum_pool = pools["msg"], pools["omega"], pools["psum"]
    nq = int(os.environ.get("KERNEL_NSWQ", "2"))

    max_wb = max(wb for _, wb in windows)
    gq = pools.setdefault("gq", [0])  # global gather counter: queue must
    # follow Tile's per-Pool-DMA-instruction DMASW lane rotation (nq | 8)
    tbl_lo, tbl_hi = tables
    do_lo = phase in (None, 0)
    do_hi = phase in (None, 1)
    mtag, otag, mtiles = "msg", "omega", K

    for w, (b0, wb) in enumerate(windows):
        nlo_tiles = wb * K_LO if do_lo else 0
        nhi_tiles = wb * K_HI if do_hi else 0
        wtiles = nlo_tiles + nhi_tiles
        msg = msg_pool.tile([P, max_wb * mtiles, TW], BF16, tag=mtag)
        omega = omega_pool.tile([P, max_wb * mtiles * P], BF16, tag=otag)
        n_lo = wb * K_LO * P
        if "nogather" in abl:
            pass
        elif gmode == "dumb":
            # diagnostic: same bytes, contiguous stream instead of gather
            nc.sync.dma_start(
                out=msg[:, 0:wtiles, :],
                in_=tbl_lo[0:wtiles * P, :].rearrange(
                    "(a b) c -> a (b c)", a=P))
        else:
            if do_lo:
                nc.gpsimd.dma_gather(
                    out_ap=msg[:, 0:nlo_tiles, :],
                    in_ap=tbl_lo,
                    idxs_ap=lo_s[:, int(lo_col_base[w]):int(lo_col_base[w]) + n_lo // 16],
                    num_idxs=n_lo,
                    num_idxs_reg=n_lo,
                    elem_size=TW,
                    queue_num=gq[0] % nq,
                    single_packet=(n_lo <= 1024),
                )
                gq[0] += 1
            if do_hi and K_HI > 0:
                n_hi = wb * K_HI * P
                nc.gpsimd.dma_gather(
                    out_ap=msg[:, nlo_tiles:nlo_tiles + wb * K_HI, :],
                    in_ap=tbl_hi,
                    idxs_ap=hi_s[:, int(hi_col_base[w]):int(hi_col_base[w]) + n_hi // 16],
                    num_idxs=n_hi,
                    num_idxs_reg=n_hi,
                    elem_size=TW,
                    queue_num=gq[0] % nq,
                    single_packet=(n_hi <= 1024),
                )
                gq[0] += 1
        gt0 = b0 * K
        ghi0 = gt0 + (wb * K_LO if do_hi else 0)  # global idx of first hi tile
        if "noomega" not in abl:
            for t in range(wtiles):
                gt = (gt0 + t) if t < nlo_tiles or phase is None else (
                    ghi0 + (t - nlo_tiles))
                nc.vector.tensor_scalar(
                    out=omega[:, t * P:(t + 1) * P],
                    in0=iota_s,
                    scalar1=dst_s[:, gt:gt + 1],
                    scalar2=ew_s[:, gt:gt + 1],
                    op0=mybir.AluOpType.is_equal,
                    op1=mybir.AluOpType.mult,
                )
        if "noflush" in abl and "nomm" in abl:
            continue
        for j in range(wb):
            brow = b0 + j
            acc = psum_pool.tile([P, P], F32, tag="acc", space="PSUM")
            if "nomm" not in abl:
                started = False
                if phase == 1:
                    nc.tensor.matmul(
                        out=acc[:], lhsT=identity_s,
                        rhs=accs[:, brow * P:(brow + 1) * P],
                        start=True, stop=(K_HI == 0))
                    started = True
                if do_lo:
                    for k in range(K_LO):
                        t = j * K_LO + k
                        nc.tensor.matmul(
                            out=acc[:], lhsT=omega[:, t * P:(t + 1) * P],
                            rhs=msg[:, t, 0:P], start=not started and k == 0,
                            stop=(phase is None and k == K - 1 and K_HI == 0)
                                 or (phase == 0 and k == K_LO - 1))
                    started = True
                if do_hi:
                    for k in range(K_HI):
                        t = nlo_tiles + j * K_HI + k
                        nc.tensor.matmul(
                            out=acc[:], lhsT=omega[:, t * P:(t + 1) * P],
                            rhs=msg[:, t, 0:P],
                            start=not started and k == 0,
                            stop=(k == K_HI - 1))
            else:
                nc.tensor.matmul(out=acc[:], lhsT=iota_s, rhs=iota_s,
                                 start=True, stop=True)
            if phase == 0:
                nc.scalar.copy(out=accs[:, brow * P:(brow + 1) * P], in_=acc[:])
            elif "noflush" not in abl:
                flush_fn(brow, acc)


def _build_program(meta, IN_CH, HID, OUT):
    pl = meta["pl"]
    nblk = meta["nblk"]
    PAD_CORE, PAD_N = meta["PAD_CORE"], meta["PAD_N"]
    HOUT = 2 * OUT
    abl = os.environ.get("KERNEL_ABL", "")

    nq = int(os.environ.get("KERNEL_NSWQ", "2"))
    scratch = int(os.environ.get("KERNEL_SCRATCH", "16384"))
    nc = bacc.Bacc(num_swdge_queues=nq, dynamic_dma_scratch_size=scratch)
    TW = int(os.environ.get("KERNEL_ESZ", "128"))
    xp_t = nc.declare_dram_parameter("xperm", [meta["PAD_T"], TW], BF16, isOutput=False)
    W1_t = nc.declare_dram_parameter("W1", [P, HID], BF16, isOutput=False)
    Wcat_t = nc.declare_dram_parameter("Wcat", [HID, HOUT], BF16, isOutput=False)
    b1_t = nc.declare_dram_parameter("b1", [1, HID], BF16, isOutput=False)
    bcat_t = nc.declare_dram_parameter("bcat", [1, HOUT], BF16, isOutput=False)
    iota_t = nc.declare_dram_parameter("iota", [P, P], BF16, isOutput=False)

    lo_t = nc.declare_dram_parameter("lo", [P, pl["lo_idx"].shape[2]], I16, isOutput=False)
    hi_t = nc.declare_dram_parameter("hi", [P, pl["hi_idx"].shape[2]], I16, isOutput=False)
    dst_t = nc.declare_dram_parameter("dst", [P, pl["TOT_TILES"]], F32, isOutput=False)
    ew_t = nc.declare_dram_parameter("ew", [P, pl["TOT_TILES"]], F32, isOutput=False)
    gmode = os.environ.get("KERNEL_GMODE", "swdge")
    ix32_t = (nc.declare_dram_parameter("ix32", [P, pl["TOT_TILES"]],
                                        mybir.dt.int32, isOutput=False)
              if gmode == "ind" else None)

    mu_t = nc.declare_dram_parameter("mu", [PAD_CORE, OUT], F32, isOutput=True)
    ls_t = nc.declare_dram_parameter("ls", [PAD_CORE, OUT], F32, isOutput=True)

    CB, nchunk, RPC = meta["CB"], meta["nchunk"], meta["RPC"]
    PAD_T = meta["PAD_T"]
    split, LOCH = meta["split"], meta["LOCH"]
    VLO_T = meta["pl"]["VLO"]
    t2mode = os.environ.get("KERNEL_T2", "shared")
    ag_ins = [
        nc.dram_tensor(f"ag_in{k}", [min(CB, nblk - k * CB) * P, HID], BF16)
        for k in range(nchunk)
    ]
    if split:
        t2sh_lo = nc.dram_tensor("t2shlo", [VLO_T, HID], BF16,
                                 addr_space="Shared")
        t2sh_hi = nc.dram_tensor("t2shhi", [PAD_T - VLO_T, HID], BF16,
                                 addr_space="Shared")
        t2lo = nc.dram_tensor("t2lo", [VLO_T, HID], BF16)
        t2hi = nc.dram_tensor("t2hi", [PAD_T - VLO_T, HID], BF16)
        table2 = table2_loc = None
    elif t2mode == "localout":
        table2 = nc.dram_tensor("table2", [PAD_T, HID], BF16)
        table2_loc = table2
    else:
        table2 = nc.dram_tensor("table2", [PAD_T, HID], BF16, addr_space="Shared")
        table2_loc = (nc.dram_tensor("table2loc", [PAD_T, HID], BF16)
                      if t2mode == "copy" else table2)

    with tile.TileContext(nc) as tc, ExitStack() as ctx:
        const = ctx.enter_context(tc.tile_pool(name="const", bufs=1))
        stage_pool = ctx.enter_context(tc.tile_pool(name="stage", bufs=3))
        msg_pool = ctx.enter_context(tc.tile_pool(name="msg", bufs=2))
        omega_pool = ctx.enter_context(tc.tile_pool(name="omega", bufs=2))
        psum_pool = ctx.enter_context(tc.tile_pool(name="psum", bufs=3, space="PSUM"))
        pr_pool = ctx.enter_context(tc.tile_pool(name="prpsum", bufs=2, space="PSUM"))
        tp_pool = ctx.enter_context(tc.tile_pool(name="tpsum", bufs=2, space="PSUM"))

        def load_const(param, shape, dtype):
            s = const.tile(shape, dtype, tag=param.name)
            nc.sync.dma_start(out=s[:], in_=param[:])
            return s[:]

        W1_s = load_const(W1_t, [P, HID], BF16)
        Wcat_s = load_const(Wcat_t, [HID, HOUT], BF16)
        b1_s = load_const(b1_t, [1, HID], BF16)
        bcat_s = load_const(bcat_t, [1, HOUT], BF16)
        iota_s = load_const(iota_t, [P, P], BF16)
        lo_s = load_const(lo_t, [P, pl["lo_idx"].shape[2]], I16)
        hi_s = load_const(hi_t, [P, pl["hi_idx"].shape[2]], I16)
        dst_s = load_const(dst_t, [P, pl["TOT_TILES"]], F32)
        ew_s = load_const(ew_t, [P, pl["TOT_TILES"]], F32)
        ix32_s = (load_const(ix32_t, [P, pl["TOT_TILES"]], mybir.dt.int32)
                  if ix32_t is not None else None)

        ones_s = const.tile([1, P], BF16, tag="ones")
        nc.vector.memset(ones_s[:], 1.0)
        identity_s = const.tile([P, P], BF16, tag="identity")
        nc.vector.memset(identity_s[:], 0.0)
        nc.gpsimd.affine_select(
            out=identity_s[:], in_=identity_s[:],
            compare_op=mybir.AluOpType.not_equal, fill=1.0,
            base=0, pattern=[[-1, P]], channel_multiplier=1)

        h1 = const.tile([P, nblk * HID], BF16, tag="h1")

        pools = dict(msg=msg_pool, omega=omega_pool, psum=psum_pool)

        def project_block(acc, Ws, bias_s, width):
            """PSUM agg [P,P] -> transpose -> @Ws + bias -> PSUM [P,width]."""
            c = stage_pool.tile([P, P], BF16, tag="pb_c")
            nc.scalar.copy(out=c[:], in_=acc[:])
            tp = tp_pool.tile([P, P], BF16, tag="pb_tp", space="PSUM")
            nc.tensor.transpose(out=tp[:], in_=c[:], identity=identity_s)
            cT = stage_pool.tile([P, P], BF16, tag="pb_cT")
            nc.scalar.copy(out=cT[:], in_=tp[:])
            pr = pr_pool.tile([P, width], F32, tag="pb_pr", space="PSUM")
            nc.tensor.matmul(out=pr[:], lhsT=cT[:], rhs=Ws, start=True, stop=False)
            nc.tensor.matmul(out=pr[:], lhsT=ones_s[:], rhs=bias_s,
                             start=False, stop=True)
            return pr

        def emit_chunk_ag(k):
            if "noAG" in abl:
                return
            rows_k = min(CB, nblk - k * CB) * P
            if split:
                if k < LOCH:
                    sh, loc, base = t2sh_lo, t2lo, k * RPC
                else:
                    sh, loc, base = t2sh_hi, t2hi, k * RPC - VLO_T
                nc.gpsimd.collective_compute(
                    "AllGather", mybir.AluOpType.bypass,
                    replica_groups=[list(range(NCORES))],
                    ins=[ag_ins[k][:]],
                    outs=[sh[base:base + rows_k * NCORES, :]])
                nc.sync.dma_start(
                    out=loc[base:base + rows_k * NCORES, :].rearrange(
                        "(a b) c -> a (b c)", a=P),
                    in_=sh[base:base + rows_k * NCORES, :].rearrange(
                        "(a b) c -> a (b c)", a=P))
                return
            nc.gpsimd.collective_compute(
                "AllGather", mybir.AluOpType.bypass,
                replica_groups=[list(range(NCORES))],
                ins=[ag_ins[k][:]],
                outs=[table2[k * RPC:k * RPC + rows_k * NCORES, :]])
            if t2mode == "copy":
                nc.sync.dma_start(
                    out=table2_loc[k * RPC:k * RPC + rows_k * NCORES, :].rearrange(
                        "(a b) c -> a (b c)", a=P),
                    in_=table2[k * RPC:k * RPC + rows_k * NCORES, :].rearrange(
                        "(a b) c -> a (b c)", a=P))

        def flush1(brow, acc):
            pr = project_block(acc, W1_s, b1_s, HID)
            nc.scalar.activation(out=h1[:, brow * HID:(brow + 1) * HID], in_=pr[:],
                                 func=mybir.ActivationFunctionType.Relu)
            k, r = brow // CB, brow % CB
            nc.sync.dma_start(out=ag_ins[k][r * P:(r + 1) * P, :],
                              in_=h1[:, brow * HID:(brow + 1) * HID])
            if brow == nblk - 1 or r == CB - 1:
                emit_chunk_ag(k)

        xp_tables = (xp_t[0:meta["pl"]["VLO"], :],
                     xp_t[meta["pl"]["VLO"]:PAD_T, :])
        if "noB" not in abl:
            _emit_pass(nc, pools, pl, xp_tables, lo_s, hi_s, dst_s, ew_s, iota_s, flush1, ix32_s=ix32_s)
        else:
            nc.vector.memset(h1[:], 0.1)
            for brow in range(nblk):
                k, r = brow // CB, brow % CB
                nc.sync.dma_start(out=ag_ins[k][r * P:(r + 1) * P, :],
                                  in_=h1[:, brow * HID:(brow + 1) * HID])
                if brow == nblk - 1 or r == CB - 1:
                    emit_chunk_ag(k)

        if not split:
            tc.strict_bb_all_engine_barrier()

        def flush2(brow, acc):
            pr = project_block(acc, Wcat_s, bcat_s, HOUT)
            o = stage_pool.tile([P, HOUT], F32, tag="otile")
            nc.scalar.copy(out=o[:], in_=pr[:])
            nc.sync.dma_start(out=mu_t[brow * P:(brow + 1) * P, :], in_=o[:, 0:OUT])
            nc.sync.dma_start(out=ls_t[brow * P:(brow + 1) * P, :], in_=o[:, OUT:HOUT])

        if "noD" not in abl:
            if split:
                accs = const.tile([P, nblk * P], BF16, tag="accs")
                _emit_pass(nc, pools, pl, (t2lo[:], None), lo_s, hi_s,
                           dst_s, ew_s, iota_s, None, phase=0, accs=accs)
                _emit_pass(nc, pools, pl, (None, t2hi[:]), lo_s, hi_s,
                           dst_s, ew_s, iota_s, flush2, phase=1, accs=accs,
                           identity_s=identity_s)
            else:
                # p2local: diagnostic — pass 2 from local xperm instead of
                # Shared table2 (wrong numerics, isolates Shared-gather cost)
                t2 = xp_t if "p2local" in abl else table2_loc
                t2_tables = (t2[0:meta["pl"]["VLO"], :],
                             t2[meta["pl"]["VLO"]:PAD_T, :])
                _emit_pass(nc, pools, pl, t2_tables, lo_s, hi_s, dst_s, ew_s,
                           iota_s, flush2, ix32_s=ix32_s)

    nc.finalize()
    return nc


# ----------------------------------------------------------------------------
# Public entry
# ----------------------------------------------------------------------------

def _prepare(x, edge_index, weight, W1, b1, Wmu, bmu, Wls, bls):
    x = np.asarray(x)
    N, IN_CH = x.shape
    HID = np.asarray(W1).shape[1]
    OUT = np.asarray(Wmu).shape[1]
    meta = _preprocess(x, np.asarray(edge_index), np.asarray(weight))
    pl = meta["pl"]

    nc = _build_program(meta, IN_CH, HID, OUT)

    TW = int(os.environ.get("KERNEL_ESZ", "128"))
    xperm = np.zeros((meta["PAD_T"], TW), np.float32)
    xperm[meta["permtab"], 0:IN_CH] = np.asarray(x, np.float32)
    Wcat = np.concatenate([np.asarray(Wmu), np.asarray(Wls)], axis=1)
    bcat = np.concatenate([np.asarray(bmu), np.asarray(bls)])
    iota = np.tile(np.arange(P, dtype=np.float32)[None, :], (P, 1))

    common = {
        "xperm": xperm.astype(NPBF16),
        "W1": np.asarray(W1, np.float32).astype(NPBF16),
        "Wcat": Wcat.astype(np.float32).astype(NPBF16),
        "b1": np.asarray(b1, np.float32).astype(NPBF16)[None, :],
        "bcat": bcat.astype(np.float32).astype(NPBF16)[None, :],
        "iota": iota.astype(NPBF16),
    }
    in_maps = []
    for c in range(NCORES):
        m = dict(common)
        m["lo"] = pl["lo_idx"][c]
        m["hi"] = pl["hi_idx"][c]
        if os.environ.get("KERNEL_GMODE", "swdge") == "ind":
            m["ix32"] = pl["idx32_slab"][c]
        m["dst"] = pl["dst_slab"][c]
        m["ew"] = pl["ew_slab"][c]
        in_maps.append(m)
    return nc, in_maps, meta


def _postprocess(results, meta):
    mu_cat = np.concatenate([results[c]["mu"] for c in range(NCORES)])
    ls_cat = np.concatenate([results[c]["ls"] for c in range(NCORES)])
    mu = mu_cat[meta["permpos"]].astype(np.float32)
    ls = ls_cat[meta["permpos"]].astype(np.float32)
    return mu, ls


def _run(x, edge_index, weight, W1, b1, Wmu, bmu, Wls, bls, trace=False):
    nc, in_maps, meta = _prepare(x, edge_index, weight, W1, b1, Wmu, bmu, Wls, bls)
    res = run_bass_kernel_spmd(nc, in_maps, list(range(NCORES)), trace=trace)
    return _postprocess(res.results, meta), res


def kernel(x, edge_index, weight, W1, b1, Wmu, bmu, Wls, bls):
    (mu, ls), _ = _run(x, edge_index, weight, W1, b1, Wmu, bmu, Wls, bls)
    return mu, ls



# revision 20
# speedup vs baseline: 3.3608x; 1.1509x over previous
"""GCN encoder (2-layer, mu/logstd heads) on 8 Trainium2 NeuronCores.

Strategy v2 (aggregate-then-project, 1D dst-partitioned graph):
  - Host: add self-loops, fold the FULL symmetric normalization into per-edge
    weights (ew = deg^-1/2[s] * w * deg^-1/2[d], f64), build a load-balancing
    node permutation (deal by in-degree into 392 blocks of 128 lanes over 8
    cores), and ONE shared edge layout used by both passes: every edge slotted
    into (core, block, lo/hi, tile, partition); wrapped-16 int16 SWDGE gather
    indices against PERMUTED node tables; per-slot dst-lane + edge-weight.
    Stage xperm = x rows permuted to table order (bf16).
  - Device (single SPMD program, TileContext):
      Pass 1: per window: dma_gather RAW x rows (bf16) -> edge-major tiles;
              omega[e,n] = (iota==dst_lane)*ew via one dual-op tensor_scalar
              per tile; PE matmuls accumulate agg[n,:] = sum_e ew*x[s] per
              128-node block in PSUM; then per block PROJECT ONCE:
              transpose(agg) -> matmul W1 -> +b1 -> ReLU -> h1 slab (SBUF,
              node-major); DMA h1 block to ag_in as soon as it's ready.
      AllGather the 8 h1 shards -> table2 (bf16, node-major, permuted order).
      Pass 2: identical windows/slots/omega against table2; per block:
              project agg2 by [Wmu||Wls] + bias, f32 out, split mu/ls DMA.
  - Host: inverse-permute rows, return (mu, logstd).

All normalization lives in ew; no per-node scaling on device. Both passes
share one index/dst/ew slab set (identical gather geometry, IN_CH==HID).
"""

import os
import sys

sys.path.insert(0, "/opt/trn_rl_repo")

import numpy as np
import ml_dtypes
from contextlib import ExitStack

import concourse.bass as bass
import concourse.bacc as bacc
import concourse.mybir as mybir
import concourse.tile as tile
from concourse.bass_utils import run_bass_kernel_spmd

P = 128
NCORES = 8
VLO = 32768          # int16 index range per gather table view
WINDOW_BLOCKS = int(os.environ.get("KERNEL_WB", "4"))

BF16 = mybir.dt.bfloat16
F32 = mybir.dt.float32
I16 = mybir.dt.int16
NPBF16 = ml_dtypes.bfloat16


def _ceil_div(a, b):
    return -(-a // b)


# ----------------------------------------------------------------------------
# Host preprocessing
# ----------------------------------------------------------------------------

def _build_pass_layout(src_rows, e_core, e_brow, e_lane, e_ew, nblk, n_table_rows,
                       vlo=VLO):
    """Slot every edge into (core, block, class, tile, partition) with
    per-block tile counts (max over cores); produce wrapped-16 int16 index
    slabs and per-slot dst-lane / edge-weight."""
    n_edges = len(src_rows)
    is_lo = src_rows < vlo
    gid = (e_core * nblk + e_brow) * 2 + (~is_lo).astype(np.int64)
    # secondary sort by src row: consecutive gather descriptors hit ascending
    # table addresses (DRAM row-buffer locality)
    order = np.argsort(gid * (1 << 17) + src_rows, kind="stable")
    gid_s = gid[order]
    counts = np.bincount(gid_s, minlength=NCORES * nblk * 2)
    starts = np.concatenate([[0], np.cumsum(counts)[:-1]])
    rank = np.arange(n_edges) - starts[gid_s]

    cnt_lo = counts[0::2].reshape(NCORES, nblk)
    cnt_hi = counts[1::2].reshape(NCORES, nblk)
    # per-block tile counts: max need across cores (program is SPMD-shared)
    kl_b = np.maximum(1, _ceil_div(cnt_lo.max(axis=0), P)).astype(np.int64)
    kh_b = (np.maximum(1, _ceil_div(cnt_hi.max(axis=0), P)).astype(np.int64)
            if cnt_hi.max() > 0 else np.zeros(nblk, np.int64))

    windows = []
    b = 0
    while b < nblk:
        wb = min(WINDOW_BLOCKS, nblk - b)
        windows.append((b, wb))
        b += wb

    # window regions: [lo tiles of wb blocks][hi tiles of wb blocks]
    nwin = len(windows)
    win_nlo = np.zeros(nwin, np.int64)
    win_nhi = np.zeros(nwin, np.int64)
    wbase = np.zeros(nwin, np.int64)
    lo_off_in_win = np.zeros(nblk, np.int64)
    hi_off_in_win = np.zeros(nblk, np.int64)
    win_of_brow = np.zeros(nblk, np.int64)
    base = 0
    for w, (b0, wb) in enumerate(windows):
        wbase[w] = base
        ol = oh = 0
        for j in range(wb):
            bb = b0 + j
            win_of_brow[bb] = w
            lo_off_in_win[bb] = ol
            hi_off_in_win[bb] = oh
            ol += kl_b[bb]
            oh += kh_b[bb]
        win_nlo[w] = ol
        win_nhi[w] = oh
        base += ol + oh
    TOT_TILES = int(base)

    e_core_s = e_core[order]
    e_brow_s = e_brow[order]
    e_lane_s = e_lane[order]
    e_ew_s = e_ew[order]
    src_s = src_rows[order]
    is_lo_s = is_lo[order]

    k_local = rank // P
    p_slot = rank % P
    w_s = win_of_brow[e_brow_s]
    t_in_reg = np.where(is_lo_s, lo_off_in_win[e_brow_s],
                        hi_off_in_win[e_brow_s]) + k_local
    gt = wbase[w_s] + np.where(is_lo_s, 0, win_nlo[w_s]) + t_in_reg

    dst_slab = np.full((NCORES, P, TOT_TILES), -1.0, np.float32)
    ew_slab = np.zeros((NCORES, P, TOT_TILES), np.float32)
    idx32_slab = np.zeros((NCORES, P, TOT_TILES), np.int32)
    dst_slab[e_core_s, p_slot, gt] = e_lane_s.astype(np.float32)
    ew_slab[e_core_s, p_slot, gt] = e_ew_s.astype(np.float32)
    idx32_slab[e_core_s, p_slot, gt] = src_s.astype(np.int32)

    lo_cols_per_win = [int(win_nlo[w]) * P // 16 for w in range(nwin)]
    hi_cols_per_win = [int(win_nhi[w]) * P // 16 for w in range(nwin)]
    lo_col_base = np.concatenate([[0], np.cumsum(lo_cols_per_win)[:-1]]).astype(np.int64)
    hi_col_base = np.concatenate([[0], np.cumsum(hi_cols_per_win)[:-1]]).astype(np.int64)
    lo_idx = np.zeros((NCORES, 16, max(1, int(sum(lo_cols_per_win)))), np.int16)
    hi_idx = np.zeros((NCORES, 16, max(1, int(sum(hi_cols_per_win)))), np.int16)

    flat_in_region = t_in_reg * P + p_slot
    col = np.where(is_lo_s, lo_col_base[w_s], hi_col_base[w_s]) + flat_in_region // 16
    row = flat_in_region % 16
    lo_mask = is_lo_s
    lo_idx[e_core_s[lo_mask], row[lo_mask], col[lo_mask]] = src_s[lo_mask].astype(np.int16)
    if kh_b.max() > 0:
        hi_mask = ~is_lo_s
        hi_idx[e_core_s[hi_mask], row[hi_mask], col[hi_mask]] = (
            (src_s[hi_mask] - vlo).astype(np.int16))

    return dict(
        VLO=vlo,
        kl_b=kl_b, kh_b=kh_b, win_nlo=win_nlo, win_nhi=win_nhi, wbase=wbase,
        lo_off_in_win=lo_off_in_win, hi_off_in_win=hi_off_in_win,
        MAXWT=int((win_nlo + win_nhi).max()),
        TOT_TILES=TOT_TILES, windows=windows,
        dst_slab=dst_slab, ew_slab=ew_slab, idx32_slab=idx32_slab,
        lo_idx=np.tile(lo_idx, (1, 8, 1)), hi_idx=np.tile(hi_idx, (1, 8, 1)),
        lo_col_base=lo_col_base, hi_col_base=hi_col_base,
        n_table_rows=n_table_rows,
    )


def _preprocess(x, edge_index, weight):
    N = x.shape[0]
    s = edge_index[0].astype(np.int64)
    d = edge_index[1].astype(np.int64)
    w = weight.astype(np.float64)
    s = np.concatenate([s, np.arange(N)])
    d = np.concatenate([d, np.arange(N)])
    w = np.concatenate([w, np.ones(N)])

    deg = np.bincount(d, weights=w, minlength=N)
    dis = np.where(deg > 0, deg ** -0.5, 0.0)
    ew = dis[s] * w * dis[d]          # full symmetric norm folded into ew

    NB = NCORES * _ceil_div(_ceil_div(N, NCORES), P)
    nblk = NB // NCORES
    PAD_CORE = nblk * P
    PAD_N = NB * P

    # balance: round-robin deal nodes (sorted by in-degree desc) into NB blocks
    tot = np.bincount(d, minlength=N)
    order = np.argsort(-tot, kind="stable")
    blk = np.empty(N, np.int64)
    lane = np.empty(N, np.int64)
    blk[order] = np.arange(N) % NB
    lane[order] = np.arange(N) // NB
    assert lane.max() < P
    core_of = blk // nblk
    brow_of = blk % nblk
    permpos = core_of * PAD_CORE + brow_of * P + lane

    # chunk-major table layout so each AllGather chunk lands contiguously:
    # row(core, brow, lane) = chunk*8*CB*P + core*CB*P + (brow%CB)*P + lane
    CB = int(os.environ.get("KERNEL_CB", "7"))
    nchunk = _ceil_div(nblk, CB)
    RPC = NCORES * CB * P           # table rows per chunk
    chunk_of = brow_of // CB
    permtab = (chunk_of * RPC + core_of * CB * P
               + (brow_of % CB) * P + lane)
    PAD_T = nchunk * RPC            # padded table rows (>= PAD_N)

    e_core = core_of[d]
    e_brow = brow_of[d]
    e_lane = lane[d]

    split = os.environ.get("KERNEL_P2SPLIT", "0") == "1"
    if split:
        c_min = _ceil_div(PAD_T - VLO, RPC)
        c_max = VLO // RPC
        loch = max(c_min, min(c_max, (nchunk + 1) // 2))
        vlo = loch * RPC
    else:
        loch, vlo = 0, VLO

    pl = _build_pass_layout(permtab[s], e_core, e_brow, e_lane, ew, nblk, PAD_T,
                            vlo=vlo)

    return dict(
        N=N, NB=NB, nblk=nblk, PAD_CORE=PAD_CORE, PAD_N=PAD_N,
        permpos=permpos, permtab=permtab, CB=CB, nchunk=nchunk, RPC=RPC,
        PAD_T=PAD_T, pl=pl, split=split, LOCH=loch,
    )


# ----------------------------------------------------------------------------
# Device program
# ----------------------------------------------------------------------------

def _emit_pass(nc, pools, pl, tables, lo_s, hi_s,
               dst_s, ew_s, iota_s, flush_fn, ix32_s=None,
               phase=None, accs=None, identity_s=None):
    """phase=None: both classes, PSUM acc per block, flush_fn(brow, acc).
    phase=0: lo class only; per block copy acc into accs slab (bf16).
    phase=1: hi class only; per block seed acc from accs via identity matmul,
             then accumulate hi tiles and flush_fn(brow, acc)."""
    abl = os.environ.get("KERNEL_ABL", "")
    gmode = os.environ.get("KERNEL_GMODE", "swdge")
    TW = int(os.environ.get("KERNEL_ESZ", "128"))  # table row width (elems)
    kl_b, kh_b = pl["kl_b"], pl["kh_b"]
    win_nlo, win_nhi, wbase = pl["win_nlo"], pl["win_nhi"], pl["wbase"]
    lo_off_in_win, hi_off_in_win = pl["lo_off_in_win"], pl["hi_off_in_win"]
    windows = pl["windows"]
    lo_col_base, hi_col_base = pl["lo_col_base"], pl["hi_col_base"]
    msg_pool, omega_pool, psum_pool = pools["msg"], pools["omega"], pools["psum"]
    nq = int(os.environ.get("KERNEL_NSWQ", "2"))

    gq = pools.setdefault("gq", [0])  # global gather counter: queue must
    # follow Tile's per-Pool-DMA-instruction DMASW lane rotation (nq | 8)
    tbl_lo, tbl_hi = tables
    do_lo = phase in (None, 0)
    do_hi = phase in (None, 1)
    MAXWT = pl["MAXWT"]

    post_window_fn = pools.get("post_window_fn")
    for w, (b0, wb) in enumerate(windows):
        nlo_tiles = int(win_nlo[w]) if do_lo else 0
        nhi_tiles = int(win_nhi[w]) if do_hi else 0
        wtiles = nlo_tiles + nhi_tiles
        msg = msg_pool.tile([P, MAXWT, TW], BF16, tag="msg")
        omega = omega_pool.tile([P, MAXWT * P], BF16, tag="omega")
        n_lo = nlo_tiles * P
        n_hi = nhi_tiles * P
        if "nogather" in abl:
            pass
        elif gmode == "dumb":
            # diagnostic: same bytes, contiguous stream instead of gather
            nc.sync.dma_start(
                out=msg[:, 0:wtiles, :],
                in_=tbl_lo[0:wtiles * P, :].rearrange(
                    "(a b) c -> a (b c)", a=P))
        else:
            if do_lo and n_lo > 0:
                nc.gpsimd.dma_gather(
                    out_ap=msg[:, 0:nlo_tiles, :],
                    in_ap=tbl_lo,
                    idxs_ap=lo_s[:, int(lo_col_base[w]):int(lo_col_base[w]) + n_lo // 16],
                    num_idxs=n_lo,
                    num_idxs_reg=n_lo,
                    elem_size=TW,
                    queue_num=gq[0] % nq,
                    single_packet=(n_lo <= 1024),
                )
                gq[0] += 1
            if do_hi and n_hi > 0:
                nc.gpsimd.dma_gather(
                    out_ap=msg[:, nlo_tiles:wtiles, :],
                    in_ap=tbl_hi,
                    idxs_ap=hi_s[:, int(hi_col_base[w]):int(hi_col_base[w]) + n_hi // 16],
                    num_idxs=n_hi,
                    num_idxs_reg=n_hi,
                    elem_size=TW,
                    queue_num=gq[0] % nq,
                    single_packet=(n_hi <= 1024),
                )
                gq[0] += 1
        # global tile id of msg tile t: lo region tiles map to wbase+t;
        # hi region tiles follow the window's (full) lo region.
        hi_gbase = int(wbase[w]) + int(win_nlo[w])
        if "noomega" not in abl:
            for t in range(wtiles):
                gt = (int(wbase[w]) + t) if t < nlo_tiles and do_lo else (
                    hi_gbase + (t - nlo_tiles))
                nc.vector.tensor_scalar(
                    out=omega[:, t * P:(t + 1) * P],
                    in0=iota_s,
                    scalar1=dst_s[:, gt:gt + 1],
                    scalar2=ew_s[:, gt:gt + 1],
                    op0=mybir.AluOpType.is_equal,
                    op1=mybir.AluOpType.mult,
                )
        if "noflush" in abl and "nomm" in abl:
            continue
        for j in range(wb):
            brow = b0 + j
            kl = int(kl_b[brow]) if do_lo else 0
            kh = int(kh_b[brow]) if do_hi else 0
            acc = psum_pool.tile([P, P], F32, tag="acc", space="PSUM")
            if "nomm" not in abl:
                started = False
                if phase == 1:
                    nc.tensor.matmul(
                        out=acc[:], lhsT=identity_s,
                        rhs=accs[:, brow * P:(brow + 1) * P],
                        start=True, stop=(kh == 0))
                    started = True
                for k in range(kl):
                    t = int(lo_off_in_win[brow]) + k
                    nc.tensor.matmul(
                        out=acc[:], lhsT=omega[:, t * P:(t + 1) * P],
                        rhs=msg[:, t, 0:P], start=not started and k == 0,
                        stop=(kh == 0 and k == kl - 1))
                if kl > 0:
                    started = True
                for k in range(kh):
                    t = nlo_tiles + int(hi_off_in_win[brow]) + k
                    nc.tensor.matmul(
                        out=acc[:], lhsT=omega[:, t * P:(t + 1) * P],
                        rhs=msg[:, t, 0:P],
                        start=not started and k == 0,
                        stop=(k == kh - 1))
            else:
                nc.tensor.matmul(out=acc[:], lhsT=iota_s, rhs=iota_s,
                                 start=True, stop=True)
            if phase == 0:
                nc.scalar.copy(out=accs[:, brow * P:(brow + 1) * P], in_=acc[:])
            elif "noflush" not in abl:
                flush_fn(brow, acc)
        if post_window_fn is not None:
            post_window_fn(w)


def _build_program(meta, IN_CH, HID, OUT):
    pl = meta["pl"]
    nblk = meta["nblk"]
    PAD_CORE, PAD_N = meta["PAD_CORE"], meta["PAD_N"]
    HOUT = 2 * OUT
    abl = os.environ.get("KERNEL_ABL", "")

    nq = int(os.environ.get("KERNEL_NSWQ", "2"))
    scratch = int(os.environ.get("KERNEL_SCRATCH", "16384"))
    nc = bacc.Bacc(num_swdge_queues=nq, dynamic_dma_scratch_size=scratch)
    TW = int(os.environ.get("KERNEL_ESZ", "128"))
    xp_t = nc.declare_dram_parameter("xperm", [meta["PAD_T"], TW], BF16, isOutput=False)
    W1_t = nc.declare_dram_parameter("W1", [P, HID], BF16, isOutput=False)
    Wcat_t = nc.declare_dram_parameter("Wcat", [HID, HOUT], BF16, isOutput=False)
    b1_t = nc.declare_dram_parameter("b1", [1, HID], BF16, isOutput=False)
    bcat_t = nc.declare_dram_parameter("bcat", [1, HOUT], BF16, isOutput=False)
    iota_t = nc.declare_dram_parameter("iota", [P, P], BF16, isOutput=False)

    lo_t = nc.declare_dram_parameter("lo", [P, pl["lo_idx"].shape[2]], I16, isOutput=False)
    hi_t = nc.declare_dram_parameter("hi", [P, pl["hi_idx"].shape[2]], I16, isOutput=False)
    dst_t = nc.declare_dram_parameter("dst", [P, pl["TOT_TILES"]], F32, isOutput=False)
    ew_t = nc.declare_dram_parameter("ew", [P, pl["TOT_TILES"]], F32, isOutput=False)
    gmode = os.environ.get("KERNEL_GMODE", "swdge")
    ix32_t = (nc.declare_dram_parameter("ix32", [P, pl["TOT_TILES"]],
                                        mybir.dt.int32, isOutput=False)
              if gmode == "ind" else None)

    mu_t = nc.declare_dram_parameter("mu", [PAD_CORE, OUT], F32, isOutput=True)
    ls_t = nc.declare_dram_parameter("ls", [PAD_CORE, OUT], F32, isOutput=True)

    CB, nchunk, RPC = meta["CB"], meta["nchunk"], meta["RPC"]
    PAD_T = meta["PAD_T"]
    split, LOCH = meta["split"], meta["LOCH"]
    VLO_T = meta["pl"]["VLO"]
    t2mode = os.environ.get("KERNEL_T2", "shared")
    ag_ins = [
        nc.dram_tensor(f"ag_in{k}", [min(CB, nblk - k * CB) * P, HID], BF16)
        for k in range(nchunk)
    ]
    if split:
        t2sh_lo = nc.dram_tensor("t2shlo", [VLO_T, HID], BF16,
                                 addr_space="Shared")
        t2sh_hi = nc.dram_tensor("t2shhi", [PAD_T - VLO_T, HID], BF16,
                                 addr_space="Shared")
        t2lo = nc.dram_tensor("t2lo", [VLO_T, HID], BF16)
        t2hi = nc.dram_tensor("t2hi", [PAD_T - VLO_T, HID], BF16)
        table2 = table2_loc = None
    elif t2mode == "localout":
        table2 = nc.dram_tensor("table2", [PAD_T, HID], BF16)
        table2_loc = table2
    else:
        table2 = nc.dram_tensor("table2", [PAD_T, HID], BF16, addr_space="Shared")
        table2_loc = (nc.dram_tensor("table2loc", [PAD_T, HID], BF16)
                      if t2mode == "copy" else table2)

    with tile.TileContext(nc) as tc, ExitStack() as ctx:
        const = ctx.enter_context(tc.tile_pool(name="const", bufs=1))
        stage_pool = ctx.enter_context(tc.tile_pool(name="stage", bufs=3))
        msg_pool = ctx.enter_context(tc.tile_pool(name="msg", bufs=2))
        omega_pool = ctx.enter_context(tc.tile_pool(name="omega", bufs=2))
        psum_pool = ctx.enter_context(tc.tile_pool(name="psum", bufs=3, space="PSUM"))
        pr_pool = ctx.enter_context(tc.tile_pool(name="prpsum", bufs=2, space="PSUM"))
        tp_pool = ctx.enter_context(tc.tile_pool(name="tpsum", bufs=2, space="PSUM"))

        def load_const(param, shape, dtype):
            s = const.tile(shape, dtype, tag=param.name)
            nc.sync.dma_start(out=s[:], in_=param[:])
            return s[:]

        W1_s = load_const(W1_t, [P, HID], BF16)
        Wcat_s = load_const(Wcat_t, [HID, HOUT], BF16)
        b1_s = load_const(b1_t, [1, HID], BF16)
        bcat_s = load_const(bcat_t, [1, HOUT], BF16)
        iota_s = load_const(iota_t, [P, P], BF16)
        lo_s = load_const(lo_t, [P, pl["lo_idx"].shape[2]], I16)
        hi_s = load_const(hi_t, [P, pl["hi_idx"].shape[2]], I16)
        dst_s = load_const(dst_t, [P, pl["TOT_TILES"]], F32)
        ew_s = load_const(ew_t, [P, pl["TOT_TILES"]], F32)
        ix32_s = (load_const(ix32_t, [P, pl["TOT_TILES"]], mybir.dt.int32)
                  if ix32_t is not None else None)

        ones_s = const.tile([1, P], BF16, tag="ones")
        nc.vector.memset(ones_s[:], 1.0)
        identity_s = const.tile([P, P], BF16, tag="identity")
        nc.vector.memset(identity_s[:], 0.0)
        nc.gpsimd.affine_select(
            out=identity_s[:], in_=identity_s[:],
            compare_op=mybir.AluOpType.not_equal, fill=1.0,
            base=0, pattern=[[-1, P]], channel_multiplier=1)

        h1 = const.tile([P, nblk * HID], BF16, tag="h1")

        pools = dict(msg=msg_pool, omega=omega_pool, psum=psum_pool)

        def project_block(acc, Ws, bias_s, width):
            """PSUM agg [P,P] -> transpose -> @Ws + bias -> PSUM [P,width]."""
            c = stage_pool.tile([P, P], BF16, tag="pb_c")
            nc.scalar.copy(out=c[:], in_=acc[:])
            tp = tp_pool.tile([P, P], BF16, tag="pb_tp", space="PSUM")
            nc.tensor.transpose(out=tp[:], in_=c[:], identity=identity_s)
            cT = stage_pool.tile([P, P], BF16, tag="pb_cT")
            nc.scalar.copy(out=cT[:], in_=tp[:])
            pr = pr_pool.tile([P, width], F32, tag="pb_pr", space="PSUM")
            nc.tensor.matmul(out=pr[:], lhsT=cT[:], rhs=Ws, start=True, stop=False)
            nc.tensor.matmul(out=pr[:], lhsT=ones_s[:], rhs=bias_s,
                             start=False, stop=True)
            return pr

        def _sh_loc_base(k):
            rows_k = min(CB, nblk - k * CB) * P
            if split:
                if k < LOCH:
                    return t2sh_lo, t2lo, k * RPC, rows_k
                return t2sh_hi, t2hi, k * RPC - VLO_T, rows_k
            return table2, table2_loc, k * RPC, rows_k

        def emit_ag(k):
            if "noAG" in abl:
                return
            sh, _, base, rows_k = _sh_loc_base(k)
            nc.gpsimd.collective_compute(
                "AllGather", mybir.AluOpType.bypass,
                replica_groups=[list(range(NCORES))],
                ins=[ag_ins[k][:]],
                outs=[sh[base:base + rows_k * NCORES, :]])

        def emit_copy(k):
            if "noAG" in abl or not (split or t2mode == "copy"):
                return
            sh, loc, base, rows_k = _sh_loc_base(k)
            nc.sync.dma_start(
                out=loc[base:base + rows_k * NCORES, :].rearrange(
                    "(a b) c -> a (b c)", a=P),
                in_=sh[base:base + rows_k * NCORES, :].rearrange(
                    "(a b) c -> a (b c)", a=P))

        # Lag AG emission a few windows behind chunk production so a
        # not-yet-ready AG never head-blocks the Pool queue (gathers).
        AGLAG = int(os.environ.get("KERNEL_AGLAG", "2"))
        WB = WINDOW_BLOCKS
        ag_emitted = [False] * nchunk

        def last_window_of_chunk(k):
            return (min((k + 1) * CB, nblk) - 1) // WB

        def post_window(w):
            for k in range(nchunk):
                if not ag_emitted[k] and last_window_of_chunk(k) <= w - AGLAG:
                    emit_ag(k)
                    ag_emitted[k] = True

        pools["post_window_fn"] = post_window

        def flush1(brow, acc):
            pr = project_block(acc, W1_s, b1_s, HID)
            nc.scalar.activation(out=h1[:, brow * HID:(brow + 1) * HID], in_=pr[:],
                                 func=mybir.ActivationFunctionType.Relu)
            k, r = brow // CB, brow % CB
            nc.sync.dma_start(out=ag_ins[k][r * P:(r + 1) * P, :],
                              in_=h1[:, brow * HID:(brow + 1) * HID])

        xp_tables = (xp_t[0:meta["pl"]["VLO"], :],
                     xp_t[meta["pl"]["VLO"]:PAD_T, :])
        if "noB" not in abl:
            _emit_pass(nc, pools, pl, xp_tables, lo_s, hi_s, dst_s, ew_s, iota_s, flush1, ix32_s=ix32_s)
        else:
            nc.vector.memset(h1[:], 0.1)
            for brow in range(nblk):
                k, r = brow // CB, brow % CB
                nc.sync.dma_start(out=ag_ins[k][r * P:(r + 1) * P, :],
                                  in_=h1[:, brow * HID:(brow + 1) * HID])
        pools.pop("post_window_fn", None)
        for k in range(nchunk):
            if not ag_emitted[k]:
                emit_ag(k)
                ag_emitted[k] = True
        # Scheduling fence: keep pass-2 instructions from being list-scheduled
        # into pass 1 (their unmet waits would head-block the in-order
        # queues). Copies come AFTER the fence so they don't extend it: each
        # waits its AG chunk; phase-0 needs only the lo chunks' copies.
        if split:
            tc.strict_bb_all_engine_barrier()
        for k in range(nchunk):
            emit_copy(k)

        if not split:
            tc.strict_bb_all_engine_barrier()

        def flush2(brow, acc):
            pr = project_block(acc, Wcat_s, bcat_s, HOUT)
            o = stage_pool.tile([P, HOUT], F32, tag="otile")
            nc.scalar.copy(out=o[:], in_=pr[:])
            nc.sync.dma_start(out=mu_t[brow * P:(brow + 1) * P, :], in_=o[:, 0:OUT])
            nc.sync.dma_start(out=ls_t[brow * P:(brow + 1) * P, :], in_=o[:, OUT:HOUT])

        if "noD" not in abl:
            if split:
                accs = const.tile([P, nblk * P], BF16, tag="accs")
                _emit_pass(nc, pools, pl, (t2lo[:], None), lo_s, hi_s,
                           dst_s, ew_s, iota_s, None, phase=0, accs=accs)
                _emit_pass(nc, pools, pl, (None, t2hi[:]), lo_s, hi_s,
                           dst_s, ew_s, iota_s, flush2, phase=1, accs=accs,
                           identity_s=identity_s)
            else:
                # p2local: diagnostic — pass 2 from local xperm instead of
                # Shared table2 (wrong numerics, isolates Shared-gather cost)
                t2 = xp_t if "p2local" in abl else table2_loc
                t2_tables = (t2[0:meta["pl"]["VLO"], :],
                             t2[meta["pl"]["VLO"]:PAD_T, :])
                _emit_pass(nc, pools, pl, t2_tables, lo_s, hi_s, dst_s, ew_s,
                           iota_s, flush2, ix32_s=ix32_s)

    nc.finalize()
    return nc


# ----------------------------------------------------------------------------
# Public entry
# ----------------------------------------------------------------------------

def _prepare(x, edge_index, weight, W1, b1, Wmu, bmu, Wls, bls):
    x = np.asarray(x)
    N, IN_CH = x.shape
    HID = np.asarray(W1).shape[1]
    OUT = np.asarray(Wmu).shape[1]
    meta = _preprocess(x, np.asarray(edge_index), np.asarray(weight))
    pl = meta["pl"]

    nc = _build_program(meta, IN_CH, HID, OUT)

    TW = int(os.environ.get("KERNEL_ESZ", "128"))
    xperm = np.zeros((meta["PAD_T"], TW), np.float32)
    xperm[meta["permtab"], 0:IN_CH] = np.asarray(x, np.float32)
    Wcat = np.concatenate([np.asarray(Wmu), np.asarray(Wls)], axis=1)
    bcat = np.concatenate([np.asarray(bmu), np.asarray(bls)])
    iota = np.tile(np.arange(P, dtype=np.float32)[None, :], (P, 1))

    common = {
        "xperm": xperm.astype(NPBF16),
        "W1": np.asarray(W1, np.float32).astype(NPBF16),
        "Wcat": Wcat.astype(np.float32).astype(NPBF16),
        "b1": np.asarray(b1, np.float32).astype(NPBF16)[None, :],
        "bcat": bcat.astype(np.float32).astype(NPBF16)[None, :],
        "iota": iota.astype(NPBF16),
    }
    in_maps = []
    for c in range(NCORES):
        m = dict(common)
        m["lo"] = pl["lo_idx"][c]
        m["hi"] = pl["hi_idx"][c]
        if os.environ.get("KERNEL_GMODE", "swdge") == "ind":
            m["ix32"] = pl["idx32_slab"][c]
        m["dst"] = pl["dst_slab"][c]
        m["ew"] = pl["ew_slab"][c]
        in_maps.append(m)
    return nc, in_maps, meta


def _postprocess(results, meta):
    mu_cat = np.concatenate([results[c]["mu"] for c in range(NCORES)])
    ls_cat = np.concatenate([results[c]["ls"] for c in range(NCORES)])
    mu = mu_cat[meta["permpos"]].astype(np.float32)
    ls = ls_cat[meta["permpos"]].astype(np.float32)
    return mu, ls


def _run(x, edge_index, weight, W1, b1, Wmu, bmu, Wls, bls, trace=False):
    nc, in_maps, meta = _prepare(x, edge_index, weight, W1, b1, Wmu, bmu, Wls, bls)
    res = run_bass_kernel_spmd(nc, in_maps, list(range(NCORES)), trace=trace)
    return _postprocess(res.results, meta), res


def kernel(x, edge_index, weight, W1, b1, Wmu, bmu, Wls, bls):
    (mu, ls), _ = _run(x, edge_index, weight, W1, b1, Wmu, bmu, Wls, bls)
    return mu, ls

